# revision 10
# baseline (speedup 1.0000x reference)
"""OTAM / DSN_TEMPORAL meta-logits kernel for 8 Trainium2 NeuronCores.

Strategy (data-parallel over queries, per sharding hint):
  - 2048 queries sharded 256/core across 8 cores; support prototypes replicated.
  - bf16 datapath end-to-end: inputs are cast to bf16 on the host (halves HBM
    traffic and PE power vs fp32r, which kept the PE HAM-throttled at 1.2GHz),
    PSUM accumulation stays fp32.
  - Frame cosine similarities via PE matmuls; frame norms via squares +
    ones-matmuls; rsqrt as Newton iteration on the DVE.
  - Core reformulation: with E = exp(-cum/lambda), lambda=0.5, the OTAM
    soft-min DP becomes a pure multiply-add recurrence
        E[l][m] = ed[l][m] * (E_diag + E_left + mask*E_up),  ed = exp(-2*d)
    with no transcendentals in the serial chain.
  - The DP runs wavefront-style over anti-diagonals ENTIRELY ON THE DVE in
    bf16: GPSIMD shares its SBUF port with the DVE, so the old DVE/GPSIMD
    lane-split made both engines ~1.7x slower; bf16 tensor_tensor gets the
    2x_1P DVE perf mode (2 elem/cycle), which beats any contended split.
  - Both DP orientations live in one unified E grid; the second orientation
    reads ed with transposed cell indices (no transposed copy).
  - The zero-pad last column (ed=1) has closed form 2*sum(col8) - last.

kernel() accepts FULL inputs and returns the FULL [2048, 64] logits.
"""

import numpy as np

# ---- problem constants (hardcoded per contest contract) ----
NCORES = 8
NQ_TOT = 2048          # total queries
NQ = NQ_TOT // NCORES  # queries per core = 256
L = 8                  # query frames
S = 8                  # support frames
D = 576                # feature dim
DPAD = 640             # padded feature dim (5 * 128)
KC = 5                 # K chunks of 128
NS = 64                # support classes
G = 2                  # query groups of 128 instances (NQ = 256 = 2*128)
NSF = NS * S           # 512 support frames
LG = L * G             # 16 (l, g) matmul groups per core
LN2 = 0.6931471805599453

# E-grid cell layout: 256 lanes = [dir(2), g(2), sj(64)]; row stride 8 cells.
# ed keeps 128-lane cells [g(2), sj(64)]; dir2 reads it with transposed cell
# indices (edT[r][scol] == ed[scol][r]), so no transposed copy is materialized.
CELL = 256
ROW = 8 * CELL         # 2048
DIAG = ROW - CELL      # 1792: (r+1, w-1) step
ECELL = 128
EROW = 8 * ECELL       # 1024
ED1 = EROW - ECELL     # 896: ed (r+1, scol-1) step for dir1
ED2 = ECELL - EROW     # dir2 reads (scol, r): step along diag = -896


def _build_program():
    import concourse.bass as bass
    import concourse.bacc as bacc
    import concourse.mybir as mybir
    import concourse.tile as tile
    from contextlib import ExitStack

    dt = mybir.dt.float32
    db = mybir.dt.bfloat16
    OP = mybir.AluOpType
    ACTF = mybir.ActivationFunctionType

    nc = bacc.Bacc("TRN2", target_bir_lowering=False, debug=False, num_devices=NCORES)

    qd = nc.dram_tensor("qslabs", [L * 128, KC * 256], db, kind="ExternalInput")
    sd = nc.dram_tensor("sfeat", [128, KC * NSF], db, kind="ExternalInput")
    od = nc.dram_tensor("logits", [NQ, NS], dt, kind="ExternalOutput")

    def V(t, off, dims):
        ap = t[:]
        return bass.AP(ap.tensor, ap.offset + off, [list(ap.ap[0])] + [list(d) for d in dims])

    def cE(r, w):          # E grid cell offset
        return r * ROW + w * CELL

    def cD(r, scol):       # ed grid cell offset (dir1 view)
        return r * EROW + scol * ECELL

    with tile.TileContext(nc) as tc:
        with ExitStack() as ctx:
            const = ctx.enter_context(tc.tile_pool(name="const", bufs=1))
            big = ctx.enter_context(tc.tile_pool(name="big", bufs=1))
            slabs = ctx.enter_context(tc.tile_pool(name="slabs", bufs=8))
            scr = ctx.enter_context(tc.tile_pool(name="scr", bufs=2))
            psm = ctx.enter_context(tc.tile_pool(name="psm", bufs=4, space="PSUM"))
            psn = ctx.enter_context(tc.tile_pool(name="psn", bufs=3, space="PSUM"))
            pss = ctx.enter_context(tc.tile_pool(name="pss", bufs=1, space="PSUM"))

            ones_nk = const.tile([128, 1], db)   # lhsT for norm matmuls (K=128, M=1)
            ones_b = const.tile([1, 128], db)    # lhsT for broadcast matmul (K=1, M=128)
            neg2 = const.tile([128, 1], dt)      # ACT bias for exp(2cos - 2)
            ln2 = const.tile([128, 1], dt)       # ACT bias ln(2) for 2/sn
            nc.gpsimd.memset(ones_nk[:], 1.0)
            nc.gpsimd.memset(ones_b[:], 1.0)
            nc.gpsimd.memset(neg2[:], -2.0)
            nc.gpsimd.memset(ln2[:], LN2)

            sT = big.tile([128, KC, NSF], db)            # bf16 support
            ed = big.tile([128, 8 * EROW], db)           # 16KB/p
            E = big.tile([128, 8 * ROW], db)             # 32KB/p DP grid
            nsq = big.tile([128, LG], dt)                # col = l*2+g
            rqn = big.tile([128, LG], dt)
            lnt = big.tile([1, NSF], dt)
            rsn2 = big.tile([1, NSF], db)
            pbs = big.tile([128, NSF], db)               # bf16 copy of 2/sn bcast
            fin = big.tile([128, 2816], db)              # final-reduction scratch
            lnsf = big.tile([128, 256], dt)              # fp32 ln outputs
            outf = big.tile([128, 128], dt)              # fp32 logits staging

            # ---------- DMAs upfront ----------
            nc.gpsimd.dma_start(sT[:], sd.ap().rearrange("p (k n) -> p k n", k=KC))
            stiles = {}
            for l in range(L):
                slab = slabs.tile([128, KC, 256], db)
                stiles[l] = slab
                nc.sync.dma_start(
                    slab[:],
                    qd.ap()[l * 128:(l + 1) * 128, :].rearrange("p (k c) -> p k c", k=KC),
                )

            # ---------- support prep (overlaps with query norm chains) ----------
            ps = pss.tile([1, NSF], dt)
            for k in range(KC):
                ssq = scr.tile([128, NSF], db, tag="sq")
                nc.scalar.square(ssq[:], sT[:, k, :])
                nc.tensor.matmul(ps[:], ones_nk[:], ssq[:],
                                 start=(k == 0), stop=(k == KC - 1))
            # rsn2 = 2/sn = exp(-0.5*ln(nsq_s) + ln2)
            nc.scalar.activation(lnt[:], ps[:], ACTF.Ln)
            nc.scalar.activation(rsn2[:], lnt[:], ACTF.Exp, bias=ln2[:1, :], scale=-0.5)
            pb = psm.tile([128, NSF], dt, tag="mm")
            nc.tensor.matmul(pb[:], ones_b[:], rsn2[:], start=True, stop=True)

            # ---------- query norm chains (squares split DVE early / ACT late) ----------
            def norm_chain(l):
                pn = psn.tile([1, 256], dt)
                for k in range(KC):
                    qsq = scr.tile([128, 256], db, tag="qsq")
                    nc.vector.tensor_tensor(qsq[:], stiles[l][:, k],
                                            stiles[l][:, k], OP.mult)
                    nc.tensor.matmul(pn[:], ones_nk[:], qsq[:],
                                     start=(k == 0), stop=(k == KC - 1))
                stg = scr.tile([1, 256], dt, tag="stg")
                nc.vector.tensor_copy(stg[:], pn[:])
                for g in range(G):
                    nc.sync.dma_start(nsq[:, l * G + g:l * G + g + 1],
                                      stg[:, g * 128:(g + 1) * 128])

            for l in range(4):
                norm_chain(l)

            # rqn = rsqrt(nsq) via Newton on DVE, in 2 half-batches so the
            # first exps unblock before the second half's norms land.
            def newton(h8):
                ya = scr.tile([128, 8], dt, tag="nta")
                yb = scr.tile([128, 8], dt, tag="ntb")
                nc.vector.tensor_scalar(rqn[:, h8], nsq[:, h8], -3.616898e-05,
                                        6.2499674e-02, OP.mult, OP.add)
                for _ in range(3):
                    nc.vector.tensor_tensor(ya[:], rqn[:, h8], rqn[:, h8], OP.mult)
                    nc.vector.tensor_tensor(yb[:], nsq[:, h8], ya[:], OP.mult)
                    nc.vector.tensor_scalar(yb[:], yb[:], -0.5, 1.5, OP.mult, OP.add)
                    nc.vector.tensor_tensor(rqn[:, h8], rqn[:, h8], yb[:], OP.mult)

            def mains(l):
                for g in range(G):
                    lg = l * G + g
                    pm = psm.tile([128, NSF], dt, tag="mm")
                    for k in range(KC):
                        nc.tensor.matmul(pm[:],
                                         stiles[l][:, k, g * 128:(g + 1) * 128],
                                         sT[:, k, :],
                                         start=(k == 0), stop=(k == KC - 1))
                    edv = V(ed, cD(l, 0) + g * 64, [[ECELL, S], [1, 64]])
                    nc.scalar.activation(
                        edv, pm[:].rearrange("p (s j) -> p s j", s=S),
                        ACTF.Exp, bias=neg2[:], scale=rqn[:, lg:lg + 1])

            newton(slice(0, 8))
            nc.vector.tensor_copy(pbs[:], pb[:])
            for k in range(KC):
                nc.vector.tensor_tensor(sT[:, k, :], sT[:, k, :], pbs[:], OP.mult)
            for l in range(4, L):
                norm_chain(l)
            newton(slice(8, 16))
            for l in range(L):
                mains(l)


            # ---------- DP wavefront (DVE-only, bf16) ----------
            # E cell (r, w) lanes: [dir*128 + g*64 + sj]. All ops on the DVE:
            # bf16 tensor_tensor hits the 2x_1P perf mode, and keeping GPSIMD
            # idle avoids the shared-SBUF-port contention.
            def edo(d, r, w):
                # ed cell offset feeding E cell (r, w): dir1 -> (r, w-1) in ed
                # grid; dir2 -> (w-1, r)
                return cD(r, w - 1) if d == 0 else cD(w - 1, r)

            for c in range(1, 16):
                # row-0 cell (r=0, w'=c-1)
                if c == 1:
                    for d in range(2):
                        nc.vector.tensor_copy(
                            V(E, cE(0, 0) + d * 128, [[1, 128]]),
                            V(ed, edo(d, 0, 1), [[1, 128]]))
                elif c <= 8:
                    for d in range(2):
                        nc.vector.tensor_tensor(
                            V(E, cE(0, c - 1) + d * 128, [[1, 128]]),
                            V(E, cE(0, c - 2) + d * 128, [[1, 128]]),
                            V(ed, edo(d, 0, c), [[1, 128]]), OP.mult)

                # masked first-column cell (r=c-1, w'=0): E = ed*(2 + E_up)
                if 2 <= c <= 8:
                    for d in range(2):
                        nc.vector.scalar_tensor_tensor(
                            V(E, cE(c - 1, 0) + d * 128, [[1, 128]]),
                            V(E, cE(c - 2, 0) + d * 128, [[1, 128]]), 2.0,
                            V(ed, edo(d, c - 1, 1), [[1, 128]]), OP.add, OP.mult)

                # interior cells r in [max(1,c-8), min(7,c-2)], w=c-r in 2..8.
                lo, hi = max(1, c - 8), min(7, c - 2)
                n = hi - lo + 1
                if n >= 1:
                    for d in range(2):
                        eds = ED1 if d == 0 else ED2
                        out = V(E, cE(lo, c - lo - 1) + d * 128, [[DIAG, n], [1, 128]])
                        nc.vector.tensor_tensor(
                            out,
                            V(E, cE(lo - 1, c - lo - 2) + d * 128, [[DIAG, n], [1, 128]]),
                            V(E, cE(lo, c - lo - 2) + d * 128, [[DIAG, n], [1, 128]]),
                            OP.add)
                        nc.vector.tensor_tensor(
                            out, out,
                            V(ed, edo(d, lo, c - lo), [[eds, n], [1, 128]]),
                            OP.mult)

                # overlap the col-7 pair reductions with the wavefront: cell
                # (r, 7) completes at diag c = r + 8 (grid col w' = 7)
                if c >= 9 and c % 2 == 1:
                    j = (c - 9) // 2
                    nc.vector.tensor_tensor(
                        V(fin, j * CELL, [[1, CELL]]),
                        V(E, cE(2 * j, 7), [[1, CELL]]),
                        V(E, cE(2 * j + 1, 7), [[1, CELL]]), OP.add)

            # ---------- final pad-column closed form + logits ----------
            # E9 = 2*sum_r E[r][7] - E[7][7], then logits = 0.5*(lnA + lnB)
            nc.vector.tensor_tensor(
                V(fin, 2048, [[CELL, 2], [1, CELL]]),
                V(fin, 0, [[2 * CELL, 2], [1, CELL]]),
                V(fin, CELL, [[2 * CELL, 2], [1, CELL]]), OP.add)
            nc.vector.tensor_tensor(
                V(fin, 1024, [[1, CELL]]),
                V(fin, 2048, [[1, CELL]]),
                V(fin, 2048 + CELL, [[1, CELL]]), OP.add)
            e9 = V(fin, 1280, [[1, CELL]])
            nc.vector.scalar_tensor_tensor(
                e9, V(fin, 1024, [[1, CELL]]), 2.0,
                V(E, cE(7, 7), [[1, CELL]]), OP.mult, OP.subtract)
            lns = lnsf[:, 0:256]
            nc.scalar.activation(lns, e9, ACTF.Ln)
            outv = outf[:, 0:128]
            nc.vector.tensor_tensor(outv, lnsf[:, 0:128],
                                    lnsf[:, 128:256], OP.add)
            nc.vector.tensor_scalar_mul(outv, outv, 0.5)
            # DMA out: logits[q = g*128 + p, sj];  src free f = g*64 + sj
            oap = od.ap()
            dst = bass.AP(oap.tensor, oap.offset, [[NS, 128], [128 * NS, G], [1, NS]])
            nc.sync.dma_start(dst, outv)

    nc.compile()
    return nc


_CACHED = None


def _get_program():
    global _CACHED
    if _CACHED is None:
        _CACHED = _build_program()
    return _CACHED


def _prep_inputs(support_features, query_features):
    """Host-side data movement: shard queries, pad D to 640, reorder layouts."""
    import ml_dtypes
    bf16 = ml_dtypes.bfloat16
    q = np.ascontiguousarray(query_features, dtype=np.float32)
    s = np.ascontiguousarray(support_features, dtype=np.float32)
    qp = np.zeros((NQ_TOT, L, DPAD), np.float32)
    qp[:, :, :D] = q
    sp = np.zeros((NSF, DPAD), np.float32)
    sp[:, :D] = s.reshape(NSF, D)
    # support frame reorder: scol = s*64 + sj  <->  frame sj*8 + s
    idx = (np.arange(NSF) % NS) * S + (np.arange(NSF) // NS)
    spr = sp[idx]                                   # [512, 640]
    sT_r = np.ascontiguousarray(
        spr.reshape(NSF, KC, 128).transpose(2, 1, 0)).reshape(128, KC * NSF).astype(bf16)
    in_maps = []
    for cidx in range(NCORES):
        qs = qp[cidx * NQ:(cidx + 1) * NQ]          # [256, 8, 640]
        q5 = qs.reshape(G, 128, L, KC, 128)          # [g, qi, l, k, dp]
        # slab for l: SBUF [128 part=dp, k, (g,qi)] -> host rows (l, dp), cols (k, g, qi)
        qT_r = np.ascontiguousarray(q5.transpose(2, 4, 3, 0, 1))  # [l, dp, k, g, qi]
        in_maps.append({
            "qslabs": qT_r.reshape(L * 128, KC * 256).astype(bf16),
            "sfeat": sT_r,
        })
    return in_maps


def kernel(support_features, query_features):
    from concourse.bass_utils import run_bass_kernel_spmd
    nc = _get_program()
    in_maps = _prep_inputs(support_features, query_features)
    res = run_bass_kernel_spmd(nc, in_maps, list(range(NCORES)))
    out = np.concatenate([res.results[i]["logits"] for i in range(NCORES)], axis=0)
    return out.astype(np.float32)
